# revision 21
# baseline (speedup 1.0000x reference)
"""Multi-head attention forward on 8 Trainium2 NeuronCores.

Problem (hardcoded): B=4, N=M=2048, D=1024, H=16, HS=64, OUT=1024, fp32.

Sharding: 8 cores = 4 batches x 2 head-groups of 8 heads. Each core
computes a partial output [2048, 1024] = sum over its 8 heads of
softmax((X_q Wq_h)(X_k Wk_h)^T / 8) (X_v Wv_h) Wo_h.  Host sums the two
head-group partials per batch and adds the projection bias.

v5 design:
  * All-fp16 PE data path: single-pass matmuls, FWL-eligible 128-col
    stationaries.
  * Phase 1 transposes run on the DMA xbar (dma_start_transpose,
    [128,1024] fp16 chunk -> [128,8,128] in one shot) -- the PE only
    does projection matmuls there.
  * Logits for the two heads of a pair run CONCURRENTLY as row-tiled
    K=64 matmuls (tile_position (0,0)/(64,0)); exp once per step over
    both heads' logits [128, 2x512] on ScalarE (the attention pacer,
    ~1000 ns/step; the PE runs ~870 ns/step).
  * Block order is n-quarter-outer / pair-inner, so each n-quarter of
    the output is fully normalized three quarters before the end; the
    output projection for quarter q is interleaved into the PE slack
    of quarter q+1's (ACT-paced) attention steps.
  * Block-tail evictions go through single merged [65, NQ] fp16 stages
    (one DVE copy frees each ctx accumulator) and fan out via DMAs:
    ctx rows to ctxn (head 1 hopping partitions 0:64 -> 64:128) and
    denominator rows to the pair's sums tile.
  * Per-(pair, n-quarter) deferred normalization (reciprocal on a f32
    scratch, fp16 broadcast via K=2 mask matmul).
"""

import os
import sys

import numpy as np

for _p in ("/opt/trn_rl_repo",):
    if _p not in sys.path and os.path.isdir(_p):
        sys.path.insert(0, _p)

B, N, M, D = 4, 2048, 2048, 1024
H, HS, OUT = 16, 64, 1024
HL = 8          # heads per core
P = 128
NPAIR = HL // 2  # head pairs per core
DT = D // P      # 8 d-tiles
NT = N // P      # 16 n-tiles
MT = M // P      # 16 m-tiles
NQ = 512         # n-quarter width per attention block
NQT = N // NQ    # 4 quarters
PIPE = 2         # ctx trails logits by PIPE m-tiles


def build_mha(tc, ins, out_ap):
    import concourse.bass as bass
    from concourse import mybir

    nc = tc.nc
    f32 = mybir.dt.float32
    f16 = mybir.dt.float16

    xq, xk, xv = ins["xq"], ins["xk"], ins["xv"]
    wq, wk, wv, wo = ins["wq"], ins["wk"], ins["wv"], ins["wo"]

    import contextlib

    with contextlib.ExitStack() as ctx:
        # ---- constant tiles ----
        const = ctx.enter_context(tc.tile_pool(name="const", bufs=1))
        identity = const.tile([P, P], f32)
        from concourse.masks import make_identity
        make_identity(nc, identity)
        identity_h = const.tile([P, P], f16)
        nc.vector.tensor_copy(identity_h[:], identity[:])
        ones_f32 = const.tile([P, HL], f32)
        nc.vector.memset(ones_f32[:], 1.0)
        ones_h = const.tile([P, HL], f16)
        nc.vector.tensor_copy(ones_h[:], ones_f32[:])

        # ---- persistent activations ----
        act_pool = ctx.enter_context(tc.tile_pool(name="acts", bufs=1))
        qt = [act_pool.tile([P, N], f16, name=f"qt{p}", tag=f"qt{p}") for p in range(NPAIR)]
        kt = [act_pool.tile([P, M], f16, name=f"kt{p}", tag=f"kt{p}") for p in range(NPAIR)]
        # V padded to 128 stationary columns (col 64 = ones for the
        # denominator, 65:128 = zeros) so the ctx LDWEIGHTS is FWL-eligible
        v_all = [act_pool.tile([P, HL, 128], f16, name=f"v{t}", tag=f"v{t}") for t in range(MT)]
        for t in range(MT):
            nc.gpsimd.memset(v_all[t][:, :, 65:128], 0.0)
        wo_sb = act_pool.tile([P, NPAIR, OUT], f16, name="wo_sb", tag="wo_sb")
        ctxn = [act_pool.tile([P, N], f16, name=f"ctxn{p}", tag=f"ctxn{p}")
                for p in range(NPAIR)]

        # ---- phase 1: weights + streamed load / xbar-transpose / project ----
        with tc.tile_pool(name="xt", bufs=2) as xt_pool, \
             tc.tile_pool(name="x_stream", bufs=4) as x_stream, \
             tc.tile_pool(name="wstage", bufs=4) as wstage_pool, \
             tc.tile_pool(name="wostage", bufs=1) as wostage_pool, \
             tc.tile_pool(name="tp_psum", bufs=4, space="PSUM") as tp_psum, \
             tc.tile_pool(name="proj_psum", bufs=4, space="PSUM") as proj_psum:

            def load_wo():
                # emitted at the END of phase 1: wo is not needed until the
                # output projection, and its DMAs otherwise gate the first
                # x-tile rounding copies on the scalar queue
                wo_stage = wostage_pool.tile(
                    [P, NPAIR, OUT], f32, name="wo_stage", tag="wost")
                for s in range(2):
                    nc.scalar.dma_start(
                        wo_stage[s * 64:(s + 1) * 64, :, :],
                        wo[s::2, :, :].rearrange("pp o d -> o pp d"))
                nc.vector.tensor_copy(wo_sb[:], wo_stage[:])

            def load_w(w_dram, pool, nm):
                # w [8, 1024, 64] -> SBUF [128(d in tile), dt, h, 64] (f16).
                # Returns (w_sb, thunks): each thunk issues one d-tile's DMA +
                # convert; the caller sprinkles them into the previous
                # stream's chunks so the transfers hide under x processing.
                w_sb = pool.tile([P, DT, HL, HS], f16, name=nm, tag=nm)

                def one(dt_i):
                    w_stage = wstage_pool.tile([P, HL, HS], f32, name="w_stage", tag="wst")
                    nc.scalar.dma_start(
                        w_stage[:],
                        w_dram[:, dt_i * P:(dt_i + 1) * P, :].rearrange("h p o -> p h o"))
                    nc.vector.tensor_copy(w_sb[:, dt_i, :, :], w_stage[:])

                return w_sb, [lambda _d=dt_i: one(_d) for dt_i in range(DT)]

            def stream_input(x_dram, w_sb, kind, prefetch=()):
                # software pipeline: transposes of chunk c+1 interleave with
                # the projection matmuls of chunk c; one next-stream weight
                # prefetch thunk is sprinkled into each early chunk.
                NC = NT // 4
                xt_tiles = {}
                prefetch = list(prefetch)

                def do_transpose(c, j):
                    if j == 0:
                        xt_tiles[c] = xt_pool.tile(
                            [P, DT, 512], f16, name="xt_c", tag="xt_c")
                    xt_c = xt_tiles[c]
                    x_t = x_stream.tile([P, D], f32, name="x_t", tag="x_t")
                    nc.sync.dma_start(
                        x_t[:], x_dram[(4 * c + j) * P:(4 * c + j + 1) * P, :])
                    # round to fp16 on the (otherwise idle in this phase)
                    # ScalarE so transpose + matmul run single-pass fp16
                    x_tr = x_stream.tile([P, D], f16, name="x_tr", tag="x_tr")
                    nc.scalar.copy(x_tr[:], x_t[:])
                    if prefetch and 4 * c + j >= 4:
                        prefetch.pop(0)()
                    for g in range(2):
                        tp = tp_psum.tile([P, 4, P], f16, name="tp", tag="tp")
                        for q in range(4):
                            dt_i = 4 * g + q
                            nc.tensor.transpose(
                                tp[:, q, :], x_tr[:, dt_i * P:(dt_i + 1) * P],
                                identity_h[:])
                        nc.vector.tensor_copy(
                            xt_c[:, 4 * g:4 * g + 4, j * P:(j + 1) * P],
                            tp[:])

                def do_proj(c, j):
                    xt_c = xt_tiles[c]
                    if kind == "v":
                        t = 4 * c + j
                        ps = proj_psum.tile([P, 512], f32, name="pp", tag="pp")
                        for dt_i in range(DT):
                            nc.tensor.matmul(
                                ps[:],
                                xt_c[:, dt_i, j * P:(j + 1) * P],
                                w_sb[:, dt_i, :, :],
                                start=(dt_i == 0), stop=(dt_i == DT - 1),
                            )
                        nc.vector.tensor_copy(
                            v_all[t][:, :, 0:64],
                            ps[:].rearrange("p (h o) -> p h o", h=HL))
                        nc.vector.tensor_copy(
                            v_all[t][:, :, 64:65],
                            ones_h[:, 0:HL].rearrange("p (h one) -> p h one", one=1))
                    else:
                        dst = qt if kind == "q" else kt
                        p = j
                        ps = proj_psum.tile([P, 512], f32, name="pp", tag="pp")
                        for dt_i in range(DT):
                            nc.tensor.matmul(
                                ps[:],
                                w_sb[:, dt_i, 2 * p:2 * p + 2, :],
                                xt_c[:, dt_i, :],
                                start=(dt_i == 0), stop=(dt_i == DT - 1),
                            )
                        nc.vector.tensor_copy(
                            dst[p][:, c * 512:(c + 1) * 512], ps[:])

                for j in range(4):
                    do_transpose(0, j)
                for c in range(NC):
                    for j in range(4):
                        if c + 1 < NC:
                            do_transpose(c + 1, j)
                        do_proj(c, j)
                    del xt_tiles[c]

            with tc.tile_pool(name="wk_pool", bufs=1) as wk_pool, \
                 tc.tile_pool(name="wv_pool", bufs=1) as wv_pool, \
                 tc.tile_pool(name="wq_pool", bufs=1) as wq_pool:
                wk_sb, wk_thunks = load_w(wk, wk_pool, "wk_sb")
                for th in wk_thunks:
                    th()
                wv_sb, wv_thunks = load_w(wv, wv_pool, "wv_sb")
                wq_sb, wq_thunks = load_w(wq, wq_pool, "wq_sb")
                stream_input(xk, wk_sb, "k", prefetch=wv_thunks)
                stream_input(xv, wv_sb, "v", prefetch=wq_thunks)
                stream_input(xq, wq_sb, "q")
            load_wo()

        # ---- phase 2: attention (quarter-outer) + interleaved out-proj ----
        # PSUM budget (8 banks): lg ring 2 x 2 banks + cps0/cps1 1 bank each
        # + norm broadcast 1 + out-proj 1 = 8.
        with tc.tile_pool(name="lgs", bufs=PIPE + 2) as lgs_pool, \
             tc.tile_pool(name="stg", bufs=3) as stg_pool, \
             tc.tile_pool(name="sums", bufs=NPAIR) as sums_pool, \
             tc.tile_pool(name="scr", bufs=2) as scr_pool, \
             tc.tile_pool(name="out_sb", bufs=2) as out_pool, \
             tc.tile_pool(name="bcast", bufs=2) as bcast_pool, \
             tc.tile_pool(name="dbounce", bufs=4, space="DRAM") as dbounce_pool, \
             tc.tile_pool(name="lg_psum", bufs=2, space="PSUM") as lg_psum, \
             tc.tile_pool(name="ctx_psum", bufs=3, space="PSUM") as ctx_psum, \
             tc.tile_pool(name="out_psum", bufs=1, space="PSUM") as out_psum:

            sumsh_pair = {}

            def get_sumsh(p):
                # one [1, N] tile per head, both at base partition 0 (the
                # GpSimd partition_broadcast requires partition-0 inputs)
                if p not in sumsh_pair:
                    sumsh_pair[p] = (
                        sums_pool.tile([1, N], f16, name=f"sh0_{p}", tag="sh0"),
                        sums_pool.tile([1, N], f16, name=f"sh1_{p}", tag="sh1"),
                    )
                return sumsh_pair[p]

            # quarter-outer, pair-inner
            blocks = [(p, nq) for nq in range(NQT) for p in range(NPAIR)]
            steps = [(bi, t) for bi in range(len(blocks)) for t in range(MT)]
            cps_map = {}
            ets = {}

            def emit_logits(bi, t):
                p, nq = blocks[bi]
                n0 = nq * NQ
                lg = lg_psum.tile([P, 2, NQ], f32, name="lg", tag="lg")
                nc.tensor.matmul(
                    lg[:, 0, :],
                    kt[p][0:64, t * P:(t + 1) * P],
                    qt[p][0:64, n0:n0 + NQ],
                    start=True, stop=True,
                )
                nc.tensor.matmul(
                    lg[:, 1, :],
                    kt[p][64:128, t * P:(t + 1) * P],
                    qt[p][64:128, n0:n0 + NQ],
                    start=True, stop=True,
                )
                et = lgs_pool.tile([P, 2, NQ], f16, name="et", tag="et")
                nc.scalar.activation(
                    et[:, :, :], lg[:, :, :],
                    mybir.ActivationFunctionType.Exp, scale=0.125)
                ets[(bi, t)] = et

            bcast_map = {}

            def emit_evict(bi):
                # reciprocal the denominator rows in place in PSUM, then one
                # merged [65, NQ] fp16 stage per head frees its ctx
                # accumulator with a single DVE copy; DMAs fan the stages out
                # to ctxn (head 1 hopping partitions) and, via a DRAM bounce,
                # broadcast the reciprocal rows for the deferred norm.
                p, nq = blocks[bi]
                n0 = nq * NQ
                cps0, cps1 = cps_map.pop(bi)
                bcast = bcast_pool.tile([P, NQ], f32, name="bcast", tag="bcast")
                bcast_map[bi] = bcast
                sts = []
                for s, cps in enumerate((cps0, cps1)):
                    st = stg_pool.tile([65, NQ], f16, name=f"st{s}", tag=f"st{s}")
                    nc.vector.tensor_copy(st[:], cps[0:65, :])
                    sts.append(st)
                for s, st in enumerate(sts):
                    nc.sync.dma_start(
                        ctxn[p][s * 64:(s + 1) * 64, n0:n0 + NQ], st[0:64, :])
                    scr = scr_pool.tile([1, NQ], f32, name="scr", tag="scr")
                    nc.vector.tensor_copy(scr[:], st[64:65, :])
                    nc.vector.reciprocal_approx_fast(scr[:], scr[:])
                    drow = dbounce_pool.tile([NQ], f32, name="drow", tag="drow")
                    nc.sync.dma_start(drow[:], scr[0:1, :])
                    drow_ap = drow[:]
                    nc.gpsimd.dma_start(
                        out=bcast[s * 64:(s + 1) * 64, :],
                        in_=bass.AP(
                            tensor=drow_ap.tensor,
                            offset=drow_ap.offset,
                            ap=[[0, 64]] + [list(x) for x in drow_ap.ap],
                        ))

            def emit_norm(bi):
                # everything (reciprocal, partition-hop, broadcast) happened
                # at evict time; the norm is one in-place multiply.
                p, nq = blocks[bi]
                n0 = nq * NQ
                bcast = bcast_map.pop(bi)
                nc.vector.tensor_mul(
                    ctxn[p][:, n0:n0 + NQ], ctxn[p][:, n0:n0 + NQ], bcast[:])

            def emit_ctx(bi, t):
                p, nq = blocks[bi]
                if t == 0:
                    cps_map[bi] = (
                        ctx_psum.tile([P, NQ], f32, name="cps0", tag="cps"),
                        ctx_psum.tile([P, NQ], f32, name="cps1", tag="cps"),
                    )
                cps0, cps1 = cps_map[bi]
                et = ets.pop((bi, t))
                nc.tensor.matmul(
                    cps0[:, :],
                    v_all[t][:, 2 * p, :],
                    et[:, 0, :],
                    start=(t == 0), stop=(t == MT - 1),
                )
                nc.tensor.matmul(
                    cps1[:, :],
                    v_all[t][:, 2 * p + 1, :],
                    et[:, 1, :],
                    start=(t == 0), stop=(t == MT - 1),
                )
                if t == MT - 1:
                    emit_evict(bi)

            # ---- interleaved output projection (micro-ops) ----
            # one micro-op = 2 accumulating matmuls of a (tile_n, c) group
            # (pairs 01 or 23); the 23-op also evicts and possibly DMAs.
            ot_map = {}

            def outproj_micro(tile_n, c, half):
                if half == 0:
                    if c == 0:
                        ot_map[tile_n] = out_pool.tile(
                            [P, OUT], f32, name="ot", tag="ot")
                    ops = out_psum.tile([P, 512], f32, name="ops", tag="ops")
                    ot_map[(tile_n, c)] = ops
                else:
                    ops = ot_map.pop((tile_n, c))
                for p in (0, 1) if half == 0 else (2, 3):
                    nc.tensor.matmul(
                        ops[:],
                        ctxn[p][:, tile_n * P:(tile_n + 1) * P],
                        wo_sb[:, p, c * 512:(c + 1) * 512],
                        start=(p == 0), stop=(p == NPAIR - 1),
                    )
                if half == 1:
                    ot = ot_map[tile_n] if c == 0 else ot_map.pop(tile_n)
                    nc.vector.tensor_copy(ot[:, c * 512:(c + 1) * 512], ops[:])
                    if c == 1:
                        nc.sync.dma_start(
                            out_ap[tile_n * P:(tile_n + 1) * P, :], ot[:])

            def quarter_micro_ops(nq):
                return [(nq * 4 + tj, c, half)
                        for tj in range(4) for c in range(2) for half in range(2)]

            outproj_queue = []
            quarter_normed = {q: 0 for q in range(NQT)}

            def note_norm(bi):
                p, nq = blocks[bi]
                quarter_normed[nq] += 1
                if quarter_normed[nq] == NPAIR:
                    outproj_queue.extend(quarter_micro_ops(nq))

            pending = []
            normed = set()
            for i, (bi, t) in enumerate(steps):
                emit_logits(bi, t)
                if i % 4 == 2 and outproj_queue:
                    outproj_micro(*outproj_queue.pop(0))
                if i >= PIPE:
                    cbi, ct = steps[i - PIPE]
                    emit_ctx(cbi, ct)
                    if ct == MT - 1:
                        pending.append((i + 4, cbi))
                while pending and pending[0][0] <= i:
                    _, nbi = pending.pop(0)
                    emit_norm(nbi)
                    normed.add(nbi)
                    note_norm(nbi)
            for i in range(len(steps) - PIPE, len(steps)):
                emit_ctx(*steps[i])
            for _, nbi in pending:
                emit_norm(nbi)
                normed.add(nbi)
                note_norm(nbi)
            for bi in range(len(blocks)):
                if bi not in normed:
                    emit_norm(bi)
                    note_norm(bi)
            while outproj_queue:
                outproj_micro(*outproj_queue.pop(0))


def build_nc():
    import concourse.bacc as bacc
    import concourse.tile as tile
    from concourse import mybir

    nc = bacc.Bacc("TRN2", target_bir_lowering=False, debug=False)
    f32 = mybir.dt.float32
    ins = {
        "xq": nc.dram_tensor("xq", (N, D), f32, kind="ExternalInput").ap(),
        "xk": nc.dram_tensor("xk", (M, D), f32, kind="ExternalInput").ap(),
        "xv": nc.dram_tensor("xv", (M, D), f32, kind="ExternalInput").ap(),
        "wq": nc.dram_tensor("wq", (HL, D, HS), f32, kind="ExternalInput").ap(),
        "wk": nc.dram_tensor("wk", (HL, D, HS), f32, kind="ExternalInput").ap(),
        "wv": nc.dram_tensor("wv", (HL, D, HS), f32, kind="ExternalInput").ap(),
        "wo": nc.dram_tensor("wo", (HL, HS, OUT), f32, kind="ExternalInput").ap(),
    }
    out_ap = nc.dram_tensor("out", (N, OUT), f32, kind="ExternalOutput").ap()
    with tile.TileContext(nc) as tc:
        build_mha(tc, ins, out_ap)
    nc.compile()
    return nc


def make_in_maps(inputs):
    q = np.ascontiguousarray(np.asarray(inputs["query"], dtype=np.float32))
    k = np.ascontiguousarray(np.asarray(inputs["key"], dtype=np.float32))
    v = np.ascontiguousarray(np.asarray(inputs["value"], dtype=np.float32))
    wq = np.asarray(inputs["query_kernel"], dtype=np.float32)
    wk = np.asarray(inputs["key_kernel"], dtype=np.float32)
    wv = np.asarray(inputs["value_kernel"], dtype=np.float32)
    wo = np.asarray(inputs["projection_kernel"], dtype=np.float32)
    in_maps = []
    for c in range(8):
        b, hg = divmod(c, 2)
        hs = slice(hg * HL, (hg + 1) * HL)
        in_maps.append({
            "xq": q[b], "xk": k[b], "xv": v[b],
            "wq": np.ascontiguousarray(wq[hs]),
            "wk": np.ascontiguousarray(wk[hs]),
            "wv": np.ascontiguousarray(wv[hs]),
            "wo": np.ascontiguousarray(wo[hs]),
        })
    return in_maps


def combine(results, bias):
    out = np.empty((B, N, OUT), dtype=np.float32)
    for b in range(B):
        out[b] = results[2 * b]["out"] + results[2 * b + 1]["out"]
    out += np.asarray(bias, dtype=np.float32)[None, None, :]
    return out


_NC_CACHE = None


def _enable_ldw_opt():
    # kept as a no-op hook for test.py compatibility
    return


def kernel(**inputs):
    global _NC_CACHE
    from concourse import bass_utils
    _enable_ldw_opt()

    if _NC_CACHE is None:
        _NC_CACHE = build_nc()
    nc = _NC_CACHE
    in_maps = make_in_maps(inputs)
    res = bass_utils.run_bass_kernel_spmd(nc, in_maps, core_ids=list(range(8)))
    return combine(res.results, inputs["projection_bias"])


# revision 22
# speedup vs baseline: 1.1461x; 1.1461x over previous
"""Multi-head attention forward on 8 Trainium2 NeuronCores.

Problem (hardcoded): B=4, N=M=2048, D=1024, H=16, HS=64, OUT=1024, fp32.

Sharding: 8 cores = 4 batches x 2 head-groups of 8 heads. Each core
computes a partial output [2048, 1024] = sum over its 8 heads of
softmax((X_q Wq_h)(X_k Wk_h)^T / 8) (X_v Wv_h) Wo_h.  Host sums the two
head-group partials per batch and adds the projection bias.

v5 design:
  * All-fp16 PE data path: single-pass matmuls, FWL-eligible 128-col
    stationaries.
  * Phase 1 transposes run on the DMA xbar (dma_start_transpose,
    [128,1024] fp16 chunk -> [128,8,128] in one shot) -- the PE only
    does projection matmuls there.
  * Logits for the two heads of a pair run CONCURRENTLY as row-tiled
    K=64 matmuls (tile_position (0,0)/(64,0)); exp once per step over
    both heads' logits [128, 2x512] on ScalarE (the attention pacer,
    ~1000 ns/step; the PE runs ~870 ns/step).
  * Block order is n-quarter-outer / pair-inner, so each n-quarter of
    the output is fully normalized three quarters before the end; the
    output projection for quarter q is interleaved into the PE slack
    of quarter q+1's (ACT-paced) attention steps.
  * Block-tail evictions go through single merged [65, NQ] fp16 stages
    (one DVE copy frees each ctx accumulator) and fan out via DMAs:
    ctx rows to ctxn (head 1 hopping partitions 0:64 -> 64:128) and
    denominator rows to the pair's sums tile.
  * Per-(pair, n-quarter) deferred normalization (reciprocal on a f32
    scratch, fp16 broadcast via K=2 mask matmul).
"""

import os
import sys

import numpy as np

for _p in ("/opt/trn_rl_repo",):
    if _p not in sys.path and os.path.isdir(_p):
        sys.path.insert(0, _p)

B, N, M, D = 4, 2048, 2048, 1024
H, HS, OUT = 16, 64, 1024
HL = 8          # heads per core
P = 128
NPAIR = HL // 2  # head pairs per core
DT = D // P      # 8 d-tiles
NT = N // P      # 16 n-tiles
MT = M // P      # 16 m-tiles
NQ = 512         # n-quarter width per attention block
NQT = N // NQ    # 4 quarters
PIPE = 2         # ctx trails logits by PIPE m-tiles


def build_mha(tc, ins, out_ap):
    import concourse.bass as bass
    from concourse import mybir

    nc = tc.nc
    f32 = mybir.dt.float32
    f16 = mybir.dt.float16

    xq, xk, xv = ins["xq"], ins["xk"], ins["xv"]
    wq, wk, wv, wo = ins["wq"], ins["wk"], ins["wv"], ins["wo"]

    import contextlib

    with contextlib.ExitStack() as ctx:
        # ---- constant tiles ----
        const = ctx.enter_context(tc.tile_pool(name="const", bufs=1))
        identity = const.tile([P, P], f32)
        from concourse.masks import make_identity
        make_identity(nc, identity)
        identity_h = const.tile([P, P], f16)
        nc.vector.tensor_copy(identity_h[:], identity[:])
        ones_f32 = const.tile([P, HL], f32)
        nc.vector.memset(ones_f32[:], 1.0)
        ones_h = const.tile([P, HL], f16)
        nc.vector.tensor_copy(ones_h[:], ones_f32[:])

        # ---- persistent activations ----
        act_pool = ctx.enter_context(tc.tile_pool(name="acts", bufs=1))
        qt = [act_pool.tile([P, N], f16, name=f"qt{p}", tag=f"qt{p}") for p in range(NPAIR)]
        kt = [act_pool.tile([P, M], f16, name=f"kt{p}", tag=f"kt{p}") for p in range(NPAIR)]
        # V padded to 128 stationary columns (col 64 = ones for the
        # denominator, 65:128 = zeros) so the ctx LDWEIGHTS is FWL-eligible
        v_all = [act_pool.tile([P, HL, 128], f16, name=f"v{t}", tag=f"v{t}") for t in range(MT)]
        for t in range(MT):
            nc.gpsimd.memset(v_all[t][:, :, 65:128], 0.0)
        wo_sb = act_pool.tile([P, NPAIR, OUT], f16, name="wo_sb", tag="wo_sb")
        ctxn = [act_pool.tile([P, N], f16, name=f"ctxn{p}", tag=f"ctxn{p}")
                for p in range(NPAIR)]

        # ---- phase 1: weights + streamed load / xbar-transpose / project ----
        with tc.tile_pool(name="xt", bufs=2) as xt_pool, \
             tc.tile_pool(name="x_stream", bufs=4) as x_stream, \
             tc.tile_pool(name="wstage", bufs=4) as wstage_pool, \
             tc.tile_pool(name="wostage", bufs=1) as wostage_pool, \
             tc.tile_pool(name="tp_psum", bufs=4, space="PSUM") as tp_psum, \
             tc.tile_pool(name="proj_psum", bufs=4, space="PSUM") as proj_psum:

            def load_wo():
                # emitted at the END of phase 1: wo is not needed until the
                # output projection, and its DMAs otherwise gate the first
                # x-tile rounding copies on the scalar queue
                wo_stage = wostage_pool.tile(
                    [P, NPAIR, OUT], f32, name="wo_stage", tag="wost")
                for s in range(2):
                    nc.scalar.dma_start(
                        wo_stage[s * 64:(s + 1) * 64, :, :],
                        wo[s::2, :, :].rearrange("pp o d -> o pp d"))
                nc.vector.tensor_copy(wo_sb[:], wo_stage[:])

            def load_w(w_dram, pool, nm):
                # w [8, 1024, 64] -> SBUF [128(d in tile), dt, h, 64] (f16).
                # Returns (w_sb, thunks): each thunk issues one d-tile's DMA +
                # convert; the caller sprinkles them into the previous
                # stream's chunks so the transfers hide under x processing.
                w_sb = pool.tile([P, DT, HL, HS], f16, name=nm, tag=nm)

                def one(dt_i):
                    w_stage = wstage_pool.tile([P, HL, HS], f32, name="w_stage", tag="wst")
                    nc.scalar.dma_start(
                        w_stage[:],
                        w_dram[:, dt_i * P:(dt_i + 1) * P, :].rearrange("h p o -> p h o"))
                    nc.vector.tensor_copy(w_sb[:, dt_i, :, :], w_stage[:])

                return w_sb, [lambda _d=dt_i: one(_d) for dt_i in range(DT)]

            def stream_input(x_dram, w_sb, kind, prefetch=()):
                # software pipeline: transposes of chunk c+1 interleave with
                # the projection matmuls of chunk c; one next-stream weight
                # prefetch thunk is sprinkled into each early chunk.
                NC = NT // 4
                xt_tiles = {}
                prefetch = list(prefetch)

                def do_transpose(c, j):
                    if j == 0:
                        xt_tiles[c] = xt_pool.tile(
                            [P, DT, 512], f16, name="xt_c", tag="xt_c")
                    xt_c = xt_tiles[c]
                    x_t = x_stream.tile([P, D], f32, name="x_t", tag="x_t")
                    nc.sync.dma_start(
                        x_t[:], x_dram[(4 * c + j) * P:(4 * c + j + 1) * P, :])
                    # round to fp16 on the (otherwise idle in this phase)
                    # ScalarE so transpose + matmul run single-pass fp16
                    x_tr = x_stream.tile([P, D], f16, name="x_tr", tag="x_tr")
                    nc.scalar.copy(x_tr[:], x_t[:])
                    if prefetch and 4 * c + j >= 4:
                        prefetch.pop(0)()
                    for g in range(2):
                        tp = tp_psum.tile([P, 4, P], f16, name="tp", tag="tp")
                        for q in range(4):
                            dt_i = 4 * g + q
                            nc.tensor.transpose(
                                tp[:, q, :], x_tr[:, dt_i * P:(dt_i + 1) * P],
                                identity_h[:])
                        nc.vector.tensor_copy(
                            xt_c[:, 4 * g:4 * g + 4, j * P:(j + 1) * P],
                            tp[:])

                def do_proj(c, j):
                    xt_c = xt_tiles[c]
                    if kind == "v":
                        t = 4 * c + j
                        ps = proj_psum.tile([P, 512], f32, name="pp", tag="pp")
                        for dt_i in range(DT):
                            nc.tensor.matmul(
                                ps[:],
                                xt_c[:, dt_i, j * P:(j + 1) * P],
                                w_sb[:, dt_i, :, :],
                                start=(dt_i == 0), stop=(dt_i == DT - 1),
                            )
                        nc.vector.tensor_copy(
                            v_all[t][:, :, 0:64],
                            ps[:].rearrange("p (h o) -> p h o", h=HL))
                        nc.vector.tensor_copy(
                            v_all[t][:, :, 64:65],
                            ones_h[:, 0:HL].rearrange("p (h one) -> p h one", one=1))
                    else:
                        dst = qt if kind == "q" else kt
                        p = j
                        ps = proj_psum.tile([P, 512], f32, name="pp", tag="pp")
                        for dt_i in range(DT):
                            nc.tensor.matmul(
                                ps[:],
                                w_sb[:, dt_i, 2 * p:2 * p + 2, :],
                                xt_c[:, dt_i, :],
                                start=(dt_i == 0), stop=(dt_i == DT - 1),
                            )
                        nc.vector.tensor_copy(
                            dst[p][:, c * 512:(c + 1) * 512], ps[:])

                for j in range(4):
                    do_transpose(0, j)
                for c in range(NC):
                    for j in range(4):
                        if c + 1 < NC:
                            do_transpose(c + 1, j)
                        do_proj(c, j)
                    del xt_tiles[c]

            with tc.tile_pool(name="wk_pool", bufs=1) as wk_pool, \
                 tc.tile_pool(name="wv_pool", bufs=1) as wv_pool, \
                 tc.tile_pool(name="wq_pool", bufs=1) as wq_pool:
                wk_sb, wk_thunks = load_w(wk, wk_pool, "wk_sb")
                for th in wk_thunks:
                    th()
                wv_sb, wv_thunks = load_w(wv, wv_pool, "wv_sb")
                wq_sb, wq_thunks = load_w(wq, wq_pool, "wq_sb")
                stream_input(xk, wk_sb, "k", prefetch=wv_thunks)
                stream_input(xv, wv_sb, "v", prefetch=wq_thunks)
                stream_input(xq, wq_sb, "q")
            load_wo()

        # ---- phase 2: attention (quarter-outer) + interleaved out-proj ----
        # PSUM budget (8 banks): lg ring 2 x 2 banks + cps0/cps1 1 bank each
        # + norm broadcast 1 + out-proj 1 = 8.
        with tc.tile_pool(name="lgs", bufs=PIPE + 2) as lgs_pool, \
             tc.tile_pool(name="stg", bufs=3) as stg_pool, \
             tc.tile_pool(name="sums", bufs=NPAIR) as sums_pool, \
             tc.tile_pool(name="scr", bufs=2) as scr_pool, \
             tc.tile_pool(name="out_sb", bufs=2) as out_pool, \
             tc.tile_pool(name="bcast", bufs=2) as bcast_pool, \
             tc.tile_pool(name="dbounce", bufs=4, space="DRAM") as dbounce_pool, \
             tc.tile_pool(name="lg_psum", bufs=2, space="PSUM") as lg_psum, \
             tc.tile_pool(name="ctx_psum", bufs=3, space="PSUM") as ctx_psum, \
             tc.tile_pool(name="out_psum", bufs=1, space="PSUM") as out_psum:

            sumsh_pair = {}

            def get_sumsh(p):
                # one [1, N] tile per head, both at base partition 0 (the
                # GpSimd partition_broadcast requires partition-0 inputs)
                if p not in sumsh_pair:
                    sumsh_pair[p] = (
                        sums_pool.tile([1, N], f16, name=f"sh0_{p}", tag="sh0"),
                        sums_pool.tile([1, N], f16, name=f"sh1_{p}", tag="sh1"),
                    )
                return sumsh_pair[p]

            # quarter-outer, pair-inner
            blocks = [(p, nq) for nq in range(NQT) for p in range(NPAIR)]
            steps = [(bi, t) for bi in range(len(blocks)) for t in range(MT)]
            cps_map = {}
            ets = {}

            def emit_logits(bi, t):
                p, nq = blocks[bi]
                n0 = nq * NQ
                lg = lg_psum.tile([P, 2, NQ], f32, name="lg", tag="lg")
                nc.tensor.matmul(
                    lg[:, 0, :],
                    kt[p][0:64, t * P:(t + 1) * P],
                    qt[p][0:64, n0:n0 + NQ],
                    start=True, stop=True,
                )
                nc.tensor.matmul(
                    lg[:, 1, :],
                    kt[p][64:128, t * P:(t + 1) * P],
                    qt[p][64:128, n0:n0 + NQ],
                    start=True, stop=True,
                )
                et = lgs_pool.tile([P, 2, NQ], f16, name="et", tag="et")
                nc.scalar.activation(
                    et[:, :, :], lg[:, :, :],
                    mybir.ActivationFunctionType.Exp, scale=0.125)
                ets[(bi, t)] = et

            bcast_map = {}

            def emit_evict(bi):
                # reciprocal the denominator rows in place in PSUM, then one
                # merged [65, NQ] fp16 stage per head frees its ctx
                # accumulator with a single DVE copy; DMAs fan the stages out
                # to ctxn (head 1 hopping partitions) and, via a DRAM bounce,
                # broadcast the reciprocal rows for the deferred norm.
                p, nq = blocks[bi]
                n0 = nq * NQ
                cps0, cps1 = cps_map.pop(bi)
                bcast = bcast_pool.tile([P, NQ], f32, name="bcast", tag="bcast")
                bcast_map[bi] = bcast
                sts = []
                for s, cps in enumerate((cps0, cps1)):
                    st = stg_pool.tile([65, NQ], f16, name=f"st{s}", tag=f"st{s}")
                    nc.vector.tensor_copy(st[:], cps[0:65, :])
                    sts.append(st)
                for s, st in enumerate(sts):
                    nc.sync.dma_start(
                        ctxn[p][s * 64:(s + 1) * 64, n0:n0 + NQ], st[0:64, :])
                    scr = scr_pool.tile([1, NQ], f32, name="scr", tag="scr")
                    nc.vector.tensor_copy(scr[:], st[64:65, :])
                    nc.vector.reciprocal_approx_fast(scr[:], scr[:])
                    drow = dbounce_pool.tile([NQ], f32, name="drow", tag="drow")
                    nc.sync.dma_start(drow[:], scr[0:1, :])
                    drow_ap = drow[:]
                    # HWDGE (sync) accepts the stride-0 DRAM source; SWDGE
                    # (gpsimd) would starve against the busy DVE on the
                    # shared port pair.
                    nc.sync.dma_start(
                        bcast[s * 64:(s + 1) * 64, :],
                        bass.AP(
                            tensor=drow_ap.tensor,
                            offset=drow_ap.offset,
                            ap=[[0, 64]] + [list(x) for x in drow_ap.ap],
                        ))

            def emit_norm(bi):
                # everything (reciprocal, partition-hop, broadcast) happened
                # at evict time; the norm is one in-place multiply.
                p, nq = blocks[bi]
                n0 = nq * NQ
                bcast = bcast_map.pop(bi)
                nc.vector.tensor_mul(
                    ctxn[p][:, n0:n0 + NQ], ctxn[p][:, n0:n0 + NQ], bcast[:])

            def emit_ctx(bi, t):
                p, nq = blocks[bi]
                if t == 0:
                    cps_map[bi] = (
                        ctx_psum.tile([P, NQ], f32, name="cps0", tag="cps"),
                        ctx_psum.tile([P, NQ], f32, name="cps1", tag="cps"),
                    )
                cps0, cps1 = cps_map[bi]
                et = ets.pop((bi, t))
                nc.tensor.matmul(
                    cps0[:, :],
                    v_all[t][:, 2 * p, :],
                    et[:, 0, :],
                    start=(t == 0), stop=(t == MT - 1),
                )
                nc.tensor.matmul(
                    cps1[:, :],
                    v_all[t][:, 2 * p + 1, :],
                    et[:, 1, :],
                    start=(t == 0), stop=(t == MT - 1),
                )
                if t == MT - 1:
                    emit_evict(bi)

            # ---- interleaved output projection (micro-ops) ----
            # one micro-op = 2 accumulating matmuls of a (tile_n, c) group
            # (pairs 01 or 23); the 23-op also evicts and possibly DMAs.
            ot_map = {}

            def outproj_micro(tile_n, c, half):
                if half == 0:
                    if c == 0:
                        ot_map[tile_n] = out_pool.tile(
                            [P, OUT], f32, name="ot", tag="ot")
                    ops = out_psum.tile([P, 512], f32, name="ops", tag="ops")
                    ot_map[(tile_n, c)] = ops
                else:
                    ops = ot_map.pop((tile_n, c))
                for p in (0, 1) if half == 0 else (2, 3):
                    nc.tensor.matmul(
                        ops[:],
                        ctxn[p][:, tile_n * P:(tile_n + 1) * P],
                        wo_sb[:, p, c * 512:(c + 1) * 512],
                        start=(p == 0), stop=(p == NPAIR - 1),
                    )
                if half == 1:
                    ot = ot_map[tile_n] if c == 0 else ot_map.pop(tile_n)
                    nc.vector.tensor_copy(ot[:, c * 512:(c + 1) * 512], ops[:])
                    if c == 1:
                        nc.sync.dma_start(
                            out_ap[tile_n * P:(tile_n + 1) * P, :], ot[:])

            def quarter_micro_ops(nq):
                return [(nq * 4 + tj, c, half)
                        for tj in range(4) for c in range(2) for half in range(2)]

            outproj_queue = []
            quarter_normed = {q: 0 for q in range(NQT)}

            def note_norm(bi):
                p, nq = blocks[bi]
                quarter_normed[nq] += 1
                if quarter_normed[nq] == NPAIR:
                    outproj_queue.extend(quarter_micro_ops(nq))

            pending = []
            normed = set()
            for i, (bi, t) in enumerate(steps):
                emit_logits(bi, t)
                if i % 4 == 2 and outproj_queue:
                    outproj_micro(*outproj_queue.pop(0))
                if i >= PIPE:
                    cbi, ct = steps[i - PIPE]
                    emit_ctx(cbi, ct)
                    if ct == MT - 1:
                        pending.append((i + 4, cbi))
                while pending and pending[0][0] <= i:
                    _, nbi = pending.pop(0)
                    emit_norm(nbi)
                    normed.add(nbi)
                    note_norm(nbi)
            for i in range(len(steps) - PIPE, len(steps)):
                emit_ctx(*steps[i])
            for _, nbi in pending:
                emit_norm(nbi)
                normed.add(nbi)
                note_norm(nbi)
            for bi in range(len(blocks)):
                if bi not in normed:
                    emit_norm(bi)
                    note_norm(bi)
            while outproj_queue:
                outproj_micro(*outproj_queue.pop(0))


def build_nc():
    import concourse.bacc as bacc
    import concourse.tile as tile
    from concourse import mybir

    nc = bacc.Bacc("TRN2", target_bir_lowering=False, debug=False)
    f32 = mybir.dt.float32
    ins = {
        "xq": nc.dram_tensor("xq", (N, D), f32, kind="ExternalInput").ap(),
        "xk": nc.dram_tensor("xk", (M, D), f32, kind="ExternalInput").ap(),
        "xv": nc.dram_tensor("xv", (M, D), f32, kind="ExternalInput").ap(),
        "wq": nc.dram_tensor("wq", (HL, D, HS), f32, kind="ExternalInput").ap(),
        "wk": nc.dram_tensor("wk", (HL, D, HS), f32, kind="ExternalInput").ap(),
        "wv": nc.dram_tensor("wv", (HL, D, HS), f32, kind="ExternalInput").ap(),
        "wo": nc.dram_tensor("wo", (HL, HS, OUT), f32, kind="ExternalInput").ap(),
    }
    out_ap = nc.dram_tensor("out", (N, OUT), f32, kind="ExternalOutput").ap()
    with tile.TileContext(nc) as tc:
        build_mha(tc, ins, out_ap)
    nc.compile()
    return nc


def make_in_maps(inputs):
    q = np.ascontiguousarray(np.asarray(inputs["query"], dtype=np.float32))
    k = np.ascontiguousarray(np.asarray(inputs["key"], dtype=np.float32))
    v = np.ascontiguousarray(np.asarray(inputs["value"], dtype=np.float32))
    wq = np.asarray(inputs["query_kernel"], dtype=np.float32)
    wk = np.asarray(inputs["key_kernel"], dtype=np.float32)
    wv = np.asarray(inputs["value_kernel"], dtype=np.float32)
    wo = np.asarray(inputs["projection_kernel"], dtype=np.float32)
    in_maps = []
    for c in range(8):
        b, hg = divmod(c, 2)
        hs = slice(hg * HL, (hg + 1) * HL)
        in_maps.append({
            "xq": q[b], "xk": k[b], "xv": v[b],
            "wq": np.ascontiguousarray(wq[hs]),
            "wk": np.ascontiguousarray(wk[hs]),
            "wv": np.ascontiguousarray(wv[hs]),
            "wo": np.ascontiguousarray(wo[hs]),
        })
    return in_maps


def combine(results, bias):
    out = np.empty((B, N, OUT), dtype=np.float32)
    for b in range(B):
        out[b] = results[2 * b]["out"] + results[2 * b + 1]["out"]
    out += np.asarray(bias, dtype=np.float32)[None, None, :]
    return out


_NC_CACHE = None


def _enable_ldw_opt():
    # kept as a no-op hook for test.py compatibility
    return


def kernel(**inputs):
    global _NC_CACHE
    from concourse import bass_utils
    _enable_ldw_opt()

    if _NC_CACHE is None:
        _NC_CACHE = build_nc()
    nc = _NC_CACHE
    in_maps = make_in_maps(inputs)
    res = bass_utils.run_bass_kernel_spmd(nc, in_maps, core_ids=list(range(8)))
    return combine(res.results, inputs["projection_bias"])


# revision 23
# speedup vs baseline: 1.1724x; 1.0230x over previous
"""Multi-head attention forward on 8 Trainium2 NeuronCores.

Problem (hardcoded): B=4, N=M=2048, D=1024, H=16, HS=64, OUT=1024, fp32.

Sharding: 8 cores = 4 batches x 2 head-groups of 8 heads. Each core
computes a partial output [2048, 1024] = sum over its 8 heads of
softmax((X_q Wq_h)(X_k Wk_h)^T / 8) (X_v Wv_h) Wo_h.  Host sums the two
head-group partials per batch and adds the projection bias.

v5 design:
  * All-fp16 PE data path: single-pass matmuls, FWL-eligible 128-col
    stationaries.
  * Phase 1 transposes run on the DMA xbar (dma_start_transpose,
    [128,1024] fp16 chunk -> [128,8,128] in one shot) -- the PE only
    does projection matmuls there.
  * Logits for the two heads of a pair run CONCURRENTLY as row-tiled
    K=64 matmuls (tile_position (0,0)/(64,0)); exp once per step over
    both heads' logits [128, 2x512] on ScalarE (the attention pacer,
    ~1000 ns/step; the PE runs ~870 ns/step).
  * Block order is n-quarter-outer / pair-inner, so each n-quarter of
    the output is fully normalized three quarters before the end; the
    output projection for quarter q is interleaved into the PE slack
    of quarter q+1's (ACT-paced) attention steps.
  * Block-tail evictions go through single merged [65, NQ] fp16 stages
    (one DVE copy frees each ctx accumulator) and fan out via DMAs:
    ctx rows to ctxn (head 1 hopping partitions 0:64 -> 64:128) and
    denominator rows to the pair's sums tile.
  * Per-(pair, n-quarter) deferred normalization (reciprocal on a f32
    scratch, fp16 broadcast via K=2 mask matmul).
"""

import os
import sys

import numpy as np

for _p in ("/opt/trn_rl_repo",):
    if _p not in sys.path and os.path.isdir(_p):
        sys.path.insert(0, _p)

B, N, M, D = 4, 2048, 2048, 1024
H, HS, OUT = 16, 64, 1024
HL = 8          # heads per core
P = 128
NPAIR = HL // 2  # head pairs per core
DT = D // P      # 8 d-tiles
NT = N // P      # 16 n-tiles
MT = M // P      # 16 m-tiles
NQ = 512         # n-quarter width per attention block
NQT = N // NQ    # 4 quarters
PIPE = 2         # ctx trails logits by PIPE m-tiles


def build_mha(tc, ins, out_ap):
    import concourse.bass as bass
    from concourse import mybir

    nc = tc.nc
    f32 = mybir.dt.float32
    f16 = mybir.dt.float16

    xq, xk, xv = ins["xq"], ins["xk"], ins["xv"]
    wq, wk, wv, wo = ins["wq"], ins["wk"], ins["wv"], ins["wo"]

    import contextlib

    with contextlib.ExitStack() as ctx:
        # ---- constant tiles ----
        const = ctx.enter_context(tc.tile_pool(name="const", bufs=1))
        identity = const.tile([P, P], f32)
        from concourse.masks import make_identity
        make_identity(nc, identity)
        identity_h = const.tile([P, P], f16)
        nc.vector.tensor_copy(identity_h[:], identity[:])
        ones_f32 = const.tile([P, HL], f32)
        nc.vector.memset(ones_f32[:], 1.0)
        ones_h = const.tile([P, HL], f16)
        nc.vector.tensor_copy(ones_h[:], ones_f32[:])

        # ---- persistent activations ----
        act_pool = ctx.enter_context(tc.tile_pool(name="acts", bufs=1))
        qt = [act_pool.tile([P, N], f16, name=f"qt{p}", tag=f"qt{p}") for p in range(NPAIR)]
        kt = [act_pool.tile([P, M], f16, name=f"kt{p}", tag=f"kt{p}") for p in range(NPAIR)]
        # V padded to 128 stationary columns (col 64 = ones for the
        # denominator, 65:128 = zeros) so the ctx LDWEIGHTS is FWL-eligible
        v_all = [act_pool.tile([P, HL, 128], f16, name=f"v{t}", tag=f"v{t}") for t in range(MT)]
        for t in range(MT):
            nc.gpsimd.memset(v_all[t][:, :, 65:128], 0.0)
        wo_sb = act_pool.tile([P, NPAIR, OUT], f16, name="wo_sb", tag="wo_sb")
        ctxn = [act_pool.tile([P, N], f16, name=f"ctxn{p}", tag=f"ctxn{p}")
                for p in range(NPAIR)]

        # ---- phase 1: weights + streamed load / xbar-transpose / project ----
        with tc.tile_pool(name="xt", bufs=2) as xt_pool, \
             tc.tile_pool(name="x_stream", bufs=4) as x_stream, \
             tc.tile_pool(name="wstage", bufs=4) as wstage_pool, \
             tc.tile_pool(name="wostage", bufs=1) as wostage_pool, \
             tc.tile_pool(name="tp_psum", bufs=4, space="PSUM") as tp_psum, \
             tc.tile_pool(name="proj_psum", bufs=4, space="PSUM") as proj_psum:

            def load_wo():
                # emitted at the END of phase 1: wo is not needed until the
                # output projection, and its DMAs otherwise gate the first
                # x-tile rounding copies on the scalar queue
                wo_stage = wostage_pool.tile(
                    [P, NPAIR, OUT], f32, name="wo_stage", tag="wost")
                for s in range(2):
                    nc.scalar.dma_start(
                        wo_stage[s * 64:(s + 1) * 64, :, :],
                        wo[s::2, :, :].rearrange("pp o d -> o pp d"))
                nc.vector.tensor_copy(wo_sb[:], wo_stage[:])

            def load_w(w_dram, pool, nm):
                # w [8, 1024, 64] -> SBUF [128(d in tile), dt, h, 64] (f16).
                # Returns (w_sb, thunks): each thunk issues one d-tile's DMA +
                # convert; the caller sprinkles them into the previous
                # stream's chunks so the transfers hide under x processing.
                w_sb = pool.tile([P, DT, HL, HS], f16, name=nm, tag=nm)

                def one(dt_i):
                    w_stage = wstage_pool.tile([P, HL, HS], f32, name="w_stage", tag="wst")
                    nc.scalar.dma_start(
                        w_stage[:],
                        w_dram[:, dt_i * P:(dt_i + 1) * P, :].rearrange("h p o -> p h o"))
                    nc.vector.tensor_copy(w_sb[:, dt_i, :, :], w_stage[:])

                return w_sb, [lambda _d=dt_i: one(_d) for dt_i in range(DT)]

            def stream_input(x_dram, w_sb, kind, prefetch=()):
                # software pipeline: transposes of chunk c+1 interleave with
                # the projection matmuls of chunk c; one next-stream weight
                # prefetch thunk is sprinkled into each early chunk.
                NC = NT // 4
                xt_tiles = {}
                prefetch = list(prefetch)

                def do_transpose(c, j):
                    if j == 0:
                        xt_tiles[c] = xt_pool.tile(
                            [P, DT, 512], f16, name="xt_c", tag="xt_c")
                    xt_c = xt_tiles[c]
                    x_t = x_stream.tile([P, D], f32, name="x_t", tag="x_t")
                    nc.sync.dma_start(
                        x_t[:], x_dram[(4 * c + j) * P:(4 * c + j + 1) * P, :])
                    # round to fp16 on the (otherwise idle in this phase)
                    # ScalarE so transpose + matmul run single-pass fp16
                    x_tr = x_stream.tile([P, D], f16, name="x_tr", tag="x_tr")
                    nc.scalar.copy(x_tr[:], x_t[:])
                    if prefetch and 4 * c + j >= 4:
                        prefetch.pop(0)()
                    for g in range(2):
                        tp = tp_psum.tile([P, 4, P], f16, name="tp", tag="tp")
                        for q in range(4):
                            dt_i = 4 * g + q
                            nc.tensor.transpose(
                                tp[:, q, :], x_tr[:, dt_i * P:(dt_i + 1) * P],
                                identity_h[:])
                        nc.vector.tensor_copy(
                            xt_c[:, 4 * g:4 * g + 4, j * P:(j + 1) * P],
                            tp[:])

                def do_proj(c, j):
                    xt_c = xt_tiles[c]
                    if kind == "v":
                        t = 4 * c + j
                        ps = proj_psum.tile([P, 512], f32, name="pp", tag="pp")
                        for dt_i in range(DT):
                            nc.tensor.matmul(
                                ps[:],
                                xt_c[:, dt_i, j * P:(j + 1) * P],
                                w_sb[:, dt_i, :, :],
                                start=(dt_i == 0), stop=(dt_i == DT - 1),
                            )
                        nc.vector.tensor_copy(
                            v_all[t][:, :, 0:64],
                            ps[:].rearrange("p (h o) -> p h o", h=HL))
                        nc.vector.tensor_copy(
                            v_all[t][:, :, 64:65],
                            ones_h[:, 0:HL].rearrange("p (h one) -> p h one", one=1))
                    else:
                        dst = qt if kind == "q" else kt
                        p = j
                        ps = proj_psum.tile([P, 512], f32, name="pp", tag="pp")
                        for dt_i in range(DT):
                            nc.tensor.matmul(
                                ps[:],
                                w_sb[:, dt_i, 2 * p:2 * p + 2, :],
                                xt_c[:, dt_i, :],
                                start=(dt_i == 0), stop=(dt_i == DT - 1),
                            )
                        nc.vector.tensor_copy(
                            dst[p][:, c * 512:(c + 1) * 512], ps[:])

                for j in range(4):
                    do_transpose(0, j)
                for c in range(NC):
                    for j in range(4):
                        if c + 1 < NC:
                            do_transpose(c + 1, j)
                        do_proj(c, j)
                    del xt_tiles[c]

            with tc.tile_pool(name="wk_pool", bufs=1) as wk_pool, \
                 tc.tile_pool(name="wv_pool", bufs=1) as wv_pool, \
                 tc.tile_pool(name="wq_pool", bufs=1) as wq_pool:
                wk_sb, wk_thunks = load_w(wk, wk_pool, "wk_sb")
                for th in wk_thunks:
                    th()
                wv_sb, wv_thunks = load_w(wv, wv_pool, "wv_sb")
                wq_sb, wq_thunks = load_w(wq, wq_pool, "wq_sb")
                stream_input(xk, wk_sb, "k", prefetch=wv_thunks)
                stream_input(xv, wv_sb, "v", prefetch=wq_thunks)
                stream_input(xq, wq_sb, "q")
            load_wo()

        # ---- phase 2: attention (quarter-outer) + interleaved out-proj ----
        # PSUM budget (8 banks): lg ring 2 x 2 banks + cps0/cps1 1 bank each
        # + norm broadcast 1 + out-proj 1 = 8.
        with tc.tile_pool(name="lgs", bufs=PIPE + 2) as lgs_pool, \
             tc.tile_pool(name="stg", bufs=3) as stg_pool, \
             tc.tile_pool(name="sums", bufs=NPAIR) as sums_pool, \
             tc.tile_pool(name="scr", bufs=2) as scr_pool, \
             tc.tile_pool(name="out_sb", bufs=2) as out_pool, \
             tc.tile_pool(name="bcast", bufs=2) as bcast_pool, \
             tc.tile_pool(name="dbounce", bufs=4, space="DRAM") as dbounce_pool, \
             tc.tile_pool(name="lg_psum", bufs=2, space="PSUM") as lg_psum, \
             tc.tile_pool(name="ctx_psum", bufs=3, space="PSUM") as ctx_psum, \
             tc.tile_pool(name="out_psum", bufs=1, space="PSUM") as out_psum:

            sumsh_pair = {}

            def get_sumsh(p):
                # one [1, N] tile per head, both at base partition 0 (the
                # GpSimd partition_broadcast requires partition-0 inputs)
                if p not in sumsh_pair:
                    sumsh_pair[p] = (
                        sums_pool.tile([1, N], f16, name=f"sh0_{p}", tag="sh0"),
                        sums_pool.tile([1, N], f16, name=f"sh1_{p}", tag="sh1"),
                    )
                return sumsh_pair[p]

            # quarter-outer, pair-inner
            blocks = [(p, nq) for nq in range(NQT) for p in range(NPAIR)]
            steps = [(bi, t) for bi in range(len(blocks)) for t in range(MT)]
            cps_map = {}
            ets = {}

            def emit_logits(bi, t):
                p, nq = blocks[bi]
                n0 = nq * NQ
                lg = lg_psum.tile([P, 2, NQ], f32, name="lg", tag="lg")
                nc.tensor.matmul(
                    lg[:, 0, :],
                    kt[p][0:64, t * P:(t + 1) * P],
                    qt[p][0:64, n0:n0 + NQ],
                    start=True, stop=True,
                )
                nc.tensor.matmul(
                    lg[:, 1, :],
                    kt[p][64:128, t * P:(t + 1) * P],
                    qt[p][64:128, n0:n0 + NQ],
                    start=True, stop=True,
                )
                et = lgs_pool.tile([P, 2, NQ], f16, name="et", tag="et")
                nc.scalar.activation(
                    et[:, :, :], lg[:, :, :],
                    mybir.ActivationFunctionType.Exp, scale=0.125)
                ets[(bi, t)] = et

            bcast_map = {}

            def emit_evict(bi):
                # reciprocal the denominator rows in place in PSUM, then one
                # merged [65, NQ] fp16 stage per head frees its ctx
                # accumulator with a single DVE copy; DMAs fan the stages out
                # to ctxn (head 1 hopping partitions) and, via a DRAM bounce,
                # broadcast the reciprocal rows for the deferred norm.
                p, nq = blocks[bi]
                n0 = nq * NQ
                cps0, cps1 = cps_map.pop(bi)
                bcast = bcast_pool.tile([P, NQ], f32, name="bcast", tag="bcast")
                bcast_map[bi] = bcast
                sts = []
                for s, cps in enumerate((cps0, cps1)):
                    st = stg_pool.tile([65, NQ], f16, name=f"st{s}", tag=f"st{s}")
                    nc.vector.tensor_copy(st[:], cps[0:65, :])
                    sts.append(st)
                for s, st in enumerate(sts):
                    nc.sync.dma_start(
                        ctxn[p][s * 64:(s + 1) * 64, n0:n0 + NQ], st[0:64, :])
                    scr = scr_pool.tile([1, NQ], f32, name="scr", tag="scr")
                    nc.vector.tensor_copy(scr[:], st[64:65, :])
                    nc.vector.reciprocal_approx_fast(scr[:], scr[:])
                    drow = dbounce_pool.tile([NQ], f32, name="drow", tag="drow")
                    nc.sync.dma_start(drow[:], scr[0:1, :])
                    drow_ap = drow[:]
                    # HWDGE (sync) accepts the stride-0 DRAM source; SWDGE
                    # (gpsimd) would starve against the busy DVE on the
                    # shared port pair.
                    nc.sync.dma_start(
                        bcast[s * 64:(s + 1) * 64, :],
                        bass.AP(
                            tensor=drow_ap.tensor,
                            offset=drow_ap.offset,
                            ap=[[0, 64]] + [list(x) for x in drow_ap.ap],
                        ))

            def emit_norm(bi):
                # everything (reciprocal, partition-hop, broadcast) happened
                # at evict time; the norm is one in-place multiply.
                p, nq = blocks[bi]
                n0 = nq * NQ
                bcast = bcast_map.pop(bi)
                nc.vector.tensor_mul(
                    ctxn[p][:, n0:n0 + NQ], ctxn[p][:, n0:n0 + NQ], bcast[:])

            def emit_ctx(bi, t):
                p, nq = blocks[bi]
                if t == 0:
                    cps_map[bi] = (
                        ctx_psum.tile([P, NQ], f32, name="cps0", tag="cps"),
                        ctx_psum.tile([P, NQ], f32, name="cps1", tag="cps"),
                    )
                cps0, cps1 = cps_map[bi]
                et = ets.pop((bi, t))
                nc.tensor.matmul(
                    cps0[:, :],
                    v_all[t][:, 2 * p, :],
                    et[:, 0, :],
                    start=(t == 0), stop=(t == MT - 1),
                )
                nc.tensor.matmul(
                    cps1[:, :],
                    v_all[t][:, 2 * p + 1, :],
                    et[:, 1, :],
                    start=(t == 0), stop=(t == MT - 1),
                )
                if t == MT - 1:
                    emit_evict(bi)

            # ---- interleaved output projection (micro-ops) ----
            # one micro-op = 2 accumulating matmuls of a (tile_n, c) group
            # (pairs 01 or 23); the 23-op also evicts and possibly DMAs.
            ot_map = {}

            def outproj_micro(tile_n, c, half):
                if half == 0:
                    if c == 0:
                        ot_map[tile_n] = out_pool.tile(
                            [P, OUT], f32, name="ot", tag="ot")
                    ops = out_psum.tile([P, 512], f32, name="ops", tag="ops")
                    ot_map[(tile_n, c)] = ops
                else:
                    ops = ot_map.pop((tile_n, c))
                for p in (0, 1) if half == 0 else (2, 3):
                    nc.tensor.matmul(
                        ops[:],
                        ctxn[p][:, tile_n * P:(tile_n + 1) * P],
                        wo_sb[:, p, c * 512:(c + 1) * 512],
                        start=(p == 0), stop=(p == NPAIR - 1),
                    )
                if half == 1:
                    ot = ot_map[tile_n] if c == 0 else ot_map.pop(tile_n)
                    nc.vector.tensor_copy(ot[:, c * 512:(c + 1) * 512], ops[:])
                    if c == 1:
                        nc.sync.dma_start(
                            out_ap[tile_n * P:(tile_n + 1) * P, :], ot[:])

            def quarter_micro_ops(nq):
                return [(nq * 4 + tj, c, half)
                        for tj in range(4) for c in range(2) for half in range(2)]

            outproj_queue = []
            quarter_normed = {q: 0 for q in range(NQT)}

            def note_norm(bi):
                p, nq = blocks[bi]
                quarter_normed[nq] += 1
                if quarter_normed[nq] == NPAIR:
                    outproj_queue.extend(quarter_micro_ops(nq))

            pending = []
            normed = set()
            for i, (bi, t) in enumerate(steps):
                emit_logits(bi, t)
                if i % 4 == 2 and outproj_queue:
                    outproj_micro(*outproj_queue.pop(0))
                if i >= PIPE:
                    cbi, ct = steps[i - PIPE]
                    emit_ctx(cbi, ct)
                    if ct == MT - 1:
                        pending.append((i + 4, cbi))
                while pending and pending[0][0] <= i:
                    _, nbi = pending.pop(0)
                    emit_norm(nbi)
                    normed.add(nbi)
                    note_norm(nbi)
            for i in range(len(steps) - PIPE, len(steps)):
                emit_ctx(*steps[i])
            for _, nbi in pending:
                emit_norm(nbi)
                normed.add(nbi)
                note_norm(nbi)
            for bi in range(len(blocks)):
                if bi not in normed:
                    emit_norm(bi)
                    note_norm(bi)
            # dead warm burst: the final norm chain (DVE + DRAM-bounce DMAs)
            # leaves the PE idle ~5-8us right before the last out-proj
            # quarter -- long enough for HAM to re-throttle the clock for
            # the whole tail. Keep the PE busy with a no-consumer
            # accumulation until the last quarter is ready.
            warm = lg_psum.tile([P, 2, NQ], f32, name="lg", tag="lg")
            for w in range(16):
                nc.tensor.matmul(
                    warm[:, 0, :],
                    kt[0][0:64, (w % MT) * P:((w % MT) + 1) * P],
                    qt[0][0:64, 0:NQ],
                    start=(w == 0), stop=(w == 15), skip_group_check=True,
                )
            while outproj_queue:
                outproj_micro(*outproj_queue.pop(0))


def build_nc():
    import concourse.bacc as bacc
    import concourse.tile as tile
    from concourse import mybir

    nc = bacc.Bacc("TRN2", target_bir_lowering=False, debug=False)
    f32 = mybir.dt.float32
    ins = {
        "xq": nc.dram_tensor("xq", (N, D), f32, kind="ExternalInput").ap(),
        "xk": nc.dram_tensor("xk", (M, D), f32, kind="ExternalInput").ap(),
        "xv": nc.dram_tensor("xv", (M, D), f32, kind="ExternalInput").ap(),
        "wq": nc.dram_tensor("wq", (HL, D, HS), f32, kind="ExternalInput").ap(),
        "wk": nc.dram_tensor("wk", (HL, D, HS), f32, kind="ExternalInput").ap(),
        "wv": nc.dram_tensor("wv", (HL, D, HS), f32, kind="ExternalInput").ap(),
        "wo": nc.dram_tensor("wo", (HL, HS, OUT), f32, kind="ExternalInput").ap(),
    }
    out_ap = nc.dram_tensor("out", (N, OUT), f32, kind="ExternalOutput").ap()
    with tile.TileContext(nc) as tc:
        build_mha(tc, ins, out_ap)
    nc.compile()
    return nc


def make_in_maps(inputs):
    q = np.ascontiguousarray(np.asarray(inputs["query"], dtype=np.float32))
    k = np.ascontiguousarray(np.asarray(inputs["key"], dtype=np.float32))
    v = np.ascontiguousarray(np.asarray(inputs["value"], dtype=np.float32))
    wq = np.asarray(inputs["query_kernel"], dtype=np.float32)
    wk = np.asarray(inputs["key_kernel"], dtype=np.float32)
    wv = np.asarray(inputs["value_kernel"], dtype=np.float32)
    wo = np.asarray(inputs["projection_kernel"], dtype=np.float32)
    in_maps = []
    for c in range(8):
        b, hg = divmod(c, 2)
        hs = slice(hg * HL, (hg + 1) * HL)
        in_maps.append({
            "xq": q[b], "xk": k[b], "xv": v[b],
            "wq": np.ascontiguousarray(wq[hs]),
            "wk": np.ascontiguousarray(wk[hs]),
            "wv": np.ascontiguousarray(wv[hs]),
            "wo": np.ascontiguousarray(wo[hs]),
        })
    return in_maps


def combine(results, bias):
    out = np.empty((B, N, OUT), dtype=np.float32)
    for b in range(B):
        out[b] = results[2 * b]["out"] + results[2 * b + 1]["out"]
    out += np.asarray(bias, dtype=np.float32)[None, None, :]
    return out


_NC_CACHE = None


def _enable_ldw_opt():
    # kept as a no-op hook for test.py compatibility
    return


def kernel(**inputs):
    global _NC_CACHE
    from concourse import bass_utils
    _enable_ldw_opt()

    if _NC_CACHE is None:
        _NC_CACHE = build_nc()
    nc = _NC_CACHE
    in_maps = make_in_maps(inputs)
    res = bass_utils.run_bass_kernel_spmd(nc, in_maps, core_ids=list(range(8)))
    return combine(res.results, inputs["projection_bias"])
